# revision 5
# baseline (speedup 1.0000x reference)
"""Trainium2 Bass kernel for nn_Attention_GATE (sparse_attention).

kernel(**inputs) takes the FULL inputs and returns (weightedContext, gate_mask)
matching reference.py. Internally: data-parallel over batch across 8 NeuronCores
(core b <-> batch b), Bass/Tile kernel per core:

  objectT  = W_obj @ object_emb           (C,W)  [setup matmuls]
  colorT   = W_col @ color_emb            (C,W)
  objT2    = objectT @ obj2color          (C,W)
  attn     = target.T @ objT2             (P,W)  [MM1, fp32, col-tiled x4]
  attn     = min(attn, maskvec)                  [masked fill]
  aw       = softmax_w(attn)                     [DVE 32x32 stream-transpose +
                                                  segmented softmax]
  wc       = aw @ colorT.T                (P,C)  [MM2, fp16, row-tiled x4]
  E2       = exp(aw); S2[w] = sum_p E2[w,p]      [gate path]
  gate     = sigmoid(784 * sum_w W_mask[w]*E2[w,p]/S2[w]
                     - sum_w W_mask[w] + b_mask)

Host side only reshapes/shards/transposes input layouts; all FLOPs on device.
"""

import sys

if "/opt/trn_rl_repo" not in sys.path:
    sys.path.insert(0, "/opt/trn_rl_repo")

import numpy as np

import concourse.bass as bass
import concourse.mybir as mybir
import concourse.tile as tile
from concourse import bacc
from concourse.alu_op_type import AluOpType

F32 = mybir.dt.float32
F16 = mybir.dt.float16

B = 8
C = 512
E = 512
Wn = 32
IH = IW = 128
P_FULL = IH * IW
NCORES = 8
MASKNEG = -30000.0
MASKPOS = 3.0e38

ACT = mybir.ActivationFunctionType
AX = mybir.AxisListType


def build_nc(P=P_FULL, num_devices=NCORES, mm1_dtype="float32", mm1_coltile=True,
             mm2_rowtile=True, f16_transpose=True, repeat=1, batch_out=True,
             batch_in=False):
    """Build the per-core Bass module. P must be a multiple of 2048."""
    SCPIX = 2048
    NSC = P // SCPIX
    assert P % SCPIX == 0

    nc = bacc.Bacc("TRN2", debug=False, target_bir_lowering=False,
                   num_devices=num_devices)

    # ---- DRAM I/O ----
    tgt = nc.dram_tensor("tgt", (C, P), F32, kind="ExternalInput").ap()
    oemb = nc.dram_tensor("oemb", (E, Wn), F32, kind="ExternalInput").ap()
    cemb4 = nc.dram_tensor("cemb4", (E, 4 * Wn), F32, kind="ExternalInput").ap()
    o2c = nc.dram_tensor("o2c", (Wn, Wn), F32, kind="ExternalInput").ap()
    wobjT = nc.dram_tensor("wobjT", (E, C), F32, kind="ExternalInput").ap()
    wcolT = nc.dram_tensor("wcolT", (E, C), F32, kind="ExternalInput").ap()
    maskvec_d = nc.dram_tensor("maskvec", (128, 1), F32, kind="ExternalInput").ap()
    ind128_d = nc.dram_tensor("ind128", (128, 128), F32, kind="ExternalInput").ap()
    ind4_d = nc.dram_tensor("ind4", (128, 4), F32, kind="ExternalInput").ap()
    wmask128_d = nc.dram_tensor("wmask128", (128, 1), F32, kind="ExternalInput").ap()
    wmaskrow4_d = nc.dram_tensor("wmaskrow4", (4, Wn), F32, kind="ExternalInput").ap()
    bmask4_d = nc.dram_tensor("bmask4", (4, 1), F32, kind="ExternalInput").ap()

    wc_out = nc.dram_tensor("wc", (P, C), F32, kind="ExternalOutput").ap()
    gate_out = nc.dram_tensor("gate", (P,), F32, kind="ExternalOutput").ap()

    mm1_dt = getattr(mybir.dt, mm1_dtype)

    with tile.TileContext(nc) as tc:
        with (
            tc.tile_pool(name="persist", bufs=1) as persist,
            tc.tile_pool(name="tgtp", bufs=(4 if batch_in else 8)) as tgtp,
            tc.tile_pool(name="work", bufs=2) as work,
            tc.tile_pool(name="e2p", bufs=NSC) as e2p,
            tc.tile_pool(name="outp", bufs=8) as outp,
            tc.tile_pool(name="gp", bufs=2) as gp,
            tc.tile_pool(name="ps1", bufs=2, space="PSUM") as ps1,
            tc.tile_pool(name="ps2", bufs=5, space="PSUM") as ps2,
            tc.tile_pool(name="pst", bufs=1, space="PSUM") as pst,
            tc.tile_pool(name="setup", bufs=1) as setup,
        ):
            # ================= setup =================
            if True:
                wobjT_sb = []
                wcolT_sb = []
                for k in range(4):
                    t1 = setup.tile([128, C], F32, tag=f"wobjT{k}")
                    nc.sync.dma_start(t1[:], wobjT[bass.ts(k, 128), :])
                    wobjT_sb.append(t1)
                    t2 = setup.tile([128, C], F32, tag=f"wcolT{k}")
                    nc.sync.dma_start(t2[:], wcolT[bass.ts(k, 128), :])
                    wcolT_sb.append(t2)
                oemb_sb = setup.tile([128, 4, Wn], F32, tag="oemb")
                nc.sync.dma_start(
                    oemb_sb[:], oemb.rearrange("(k p) w -> p k w", p=128))
                cemb4_sb = setup.tile([128, 4, 4 * Wn], F32, tag="cemb4")
                nc.sync.dma_start(
                    cemb4_sb[:], cemb4.rearrange("(k p) w -> p k w", p=128))
                o2c_sb = setup.tile([Wn, Wn], F32, tag="o2c")
                nc.sync.dma_start(o2c_sb[:], o2c[:, :])

                # objectT^T (W, C) = (W_obj @ object_emb)^T
                ps_oTT = ps2.tile([Wn, C], F32, tag="psmm2")
                for k in range(4):
                    nc.tensor.matmul(ps_oTT[:], oemb_sb[:, k, :], wobjT_sb[k][:],
                                     start=(k == 0), stop=(k == 3))
                oTT_sb = setup.tile([Wn, C], F32, tag="oTT")
                nc.vector.tensor_copy(oTT_sb[:], ps_oTT[:])

                # colorT^T replicated x4 on partitions: (4*W, C)
                ps_cTT = ps2.tile([128, C], F32, tag="psmm2")
                for k in range(4):
                    nc.tensor.matmul(ps_cTT[:], cemb4_sb[:, k, :], wcolT_sb[k][:],
                                     start=(k == 0), stop=(k == 3))
                cTT4_sb = persist.tile([128, C], F16, tag="cTT4")
                nc.vector.tensor_copy(cTT4_sb[:], ps_cTT[:])

                # objT2 (C, W) chunk-major: oT2_sb[:, 32k:32k+32] = c-chunk k
                ps_oT2 = ps2.tile([128, 128], F32, tag="psmm2")
                for k in range(4):
                    nc.tensor.matmul(ps_oT2[:, bass.ts(k, Wn)],
                                     oTT_sb[:, bass.ts(k, 128)], o2c_sb[:],
                                     start=True, stop=True)
                oT2_sb = persist.tile([128, 128], F32, tag="oT2")
                nc.vector.tensor_copy(oT2_sb[:], ps_oT2[:])
                if mm1_dtype != "float32":
                    oT2_mm = persist.tile([128, 128], mm1_dt, tag="oT2mm")
                    nc.vector.tensor_copy(oT2_mm[:], ps_oT2[:])
                else:
                    oT2_mm = oT2_sb

            # constants
            maskvec = persist.tile([128, 1], F32, tag="maskvec")
            nc.sync.dma_start(maskvec[:], maskvec_d[:, :])
            ind128 = persist.tile([128, 128], F32, tag="ind128")
            nc.sync.dma_start(ind128[:], ind128_d[:, :])
            ind4 = persist.tile([128, 4], F32, tag="ind4")
            nc.sync.dma_start(ind4[:], ind4_d[:, :])
            wmask128 = persist.tile([128, 1], F32, tag="wmask128")
            nc.sync.dma_start(wmask128[:], wmask128_d[:, :])
            wmaskrow4 = persist.tile([4, Wn], F32, tag="wmaskrow4")
            nc.sync.dma_start(wmaskrow4[:], wmaskrow4_d[:, :])
            bmask4 = persist.tile([4, 1], F32, tag="bmask4")
            nc.sync.dma_start(bmask4[:], bmask4_d[:, :])

            wc_q = wc_out.rearrange("(s j t p) c -> s t p j c", j=4, t=4, p=128)
            for _rep in range(repeat):
              s2all = persist.tile([128, NSC], F32, tag="s2all")
              e2_tiles = []

              # ================= main loop =================
              for sc in range(NSC):
                  # load 4 c-chunks of target for this 2048-pixel super-chunk
                  if batch_in:
                      tgt_pair = []
                      for h in range(2):
                          tp = tgtp.tile([128, 2, SCPIX], F32, tag="tgtb")
                          nc.sync.dma_start(
                              tp[:],
                              tgt.rearrange("(k p) x -> p k x", p=128)
                              [:, bass.ds(2 * h, 2), bass.ts(sc, SCPIX)])
                          tgt_pair.append(tp)
                      tgt_t = [tgt_pair[k // 2][:, k % 2, :] for k in range(4)]
                  else:
                      tgt_t = []
                      for k in range(4):
                          t = tgtp.tile([128, SCPIX], F32, tag="tgt")
                          nc.sync.dma_start(
                              t[:], tgt[bass.ts(k, 128), bass.ts(sc, SCPIX)])
                          tgt_t.append(t[:])
                  if mm1_dtype == "float16":
                      cv = []
                      for k in range(4):
                          t = tgtp.tile([128, SCPIX], F16, tag="tgt16")
                          nc.vector.tensor_copy(t[:], tgt_t[k])
                          cv.append(t)
                      tgt_mm = cv
                  else:
                      tgt_mm = tgt_t

                  # MM1: logits, col-tiled x4 (pixel chunk g -> col group g)
                  p1 = ps1.tile([128, 512], F32, tag="ps1")
                  if mm1_coltile:
                      for k in range(4):
                          for g in range(4):
                              nc.tensor.matmul(
                                  p1[bass.ds(32 * g, 32), :],
                                  oT2_mm[:, bass.ts(k, Wn)],
                                  tgt_mm[k][:, bass.ts(g, 512)],
                                  start=(k == 0), stop=(k == 3),
                                  skip_group_check=True,
                                  tile_position=(0, 32 * g))
                  else:
                      for g in range(4):
                          for k in range(4):
                              nc.tensor.matmul(
                                  p1[bass.ds(32 * g, 32), :],
                                  oT2_mm[:, bass.ts(k, Wn)],
                                  tgt_mm[k][:, bass.ts(g, 512)],
                                  start=(k == 0), stop=(k == 3),
                                  skip_group_check=True)

                  # drain + masked fill (per-partition word mask)
                  s_sb = work.tile([128, 512], F32, tag="s_sb")
                  nc.vector.tensor_scalar(s_sb[:], p1[:], maskvec[:, 0:1], None,
                                          AluOpType.min)

                  # transpose to pixel-major (32x32 blocks)
                  t_sb = work.tile([128, 512], F32, tag="t_sb")
                  nc.vector.transpose(t_sb[:], s_sb[:])
                  t3 = t_sb[:].rearrange("p (s w) -> p s w", w=Wn)

                  # segmented softmax over words
                  mx = work.tile([128, 16], F32, tag="mx")
                  nc.vector.reduce_max(mx[:], t3, axis=AX.X)
                  tsub = work.tile([128, 512], F16, tag="tsub")
                  mxb = mx[:].unsqueeze(2).broadcast_to((128, 16, Wn))
                  nc.vector.tensor_tensor(
                      tsub[:].rearrange("p (s w) -> p s w", w=Wn), t3, mxb,
                      AluOpType.subtract)
                  ex = work.tile([128, 512], F16, tag="ex")
                  nc.scalar.activation(ex[:], tsub[:], ACT.Exp)
                  sm = work.tile([128, 16], F32, tag="sm")
                  nc.vector.reduce_sum(
                      sm[:], ex[:].rearrange("p (s w) -> p s w", w=Wn), axis=AX.X)
                  rc = work.tile([128, 16], F32, tag="rc")
                  nc.vector.reciprocal(rc[:], sm[:])
                  awt = work.tile([128, 512], F16, tag="awt")
                  rcb = rc[:].unsqueeze(2).broadcast_to((128, 16, Wn))
                  nc.vector.tensor_tensor(
                      awt[:].rearrange("p (s w) -> p s w", w=Wn),
                      ex[:].rearrange("p (s w) -> p s w", w=Wn), rcb,
                      AluOpType.mult)

                  # transpose back to word-major for MM2 stationary
                  if f16_transpose:
                      aws = work.tile([128, 512], F16, tag="aws")
                      nc.vector.transpose(aws[:], awt[:])
                  else:
                      awt32 = work.tile([128, 512], F32, tag="awt32")
                      nc.vector.tensor_copy(awt32[:], awt[:])
                      aws32 = work.tile([128, 512], F32, tag="aws32")
                      nc.vector.transpose(aws32[:], awt32[:])
                      aws = work.tile([128, 512], F16, tag="aws")
                      nc.vector.tensor_copy(aws[:], aws32[:])

                  # gate path: E2 = exp(aw), accumulate per-word pixel sums
                  e2 = e2p.tile([128, 512], F16, tag="e2")
                  nc.scalar.activation(e2[:], aws[:], ACT.Exp)
                  e2_tiles.append(e2)
                  nc.vector.reduce_sum(s2all[:, sc:sc + 1], e2[:], axis=AX.X)

                  # MM2: wc tiles (128 px, 512 c), row-tiled x4
                  for t in range(4):
                      if batch_out:
                          o_sb = outp.tile([128, 4, 512], F32, tag="out")
                      for j in range(4):
                          p2 = ps2.tile([128, 512], F32, tag="psmm2")
                          if mm2_rowtile:
                              nc.tensor.matmul(
                                  p2[:], aws[bass.ds(32 * j, 32), bass.ts(t, 128)],
                                  cTT4_sb[bass.ds(32 * j, 32), :],
                                  start=True, stop=True,
                                  tile_position=(32 * j, 0))
                          else:
                              nc.tensor.matmul(
                                  p2[:], aws[bass.ds(32 * j, 32), bass.ts(t, 128)],
                                  cTT4_sb[bass.ds(32 * j, 32), :],
                                  start=True, stop=True)
                          if batch_out:
                              dst = o_sb[:, j, :]
                          else:
                              o_sb1 = outp.tile([128, 512], F32, tag="out")
                              dst = o_sb1[:]
                          if (t * 4 + j) % 2 == 0:
                              nc.vector.tensor_copy(dst, p2[:])
                          else:
                              nc.scalar.copy(dst, p2[:])
                          if not batch_out:
                              pixbase = sc * SCPIX + 512 * j + 128 * t
                              nc.sync.dma_start(
                                  wc_out[bass.ds(pixbase, 128), :], o_sb1[:])
                      if batch_out:
                          nc.sync.dma_start(wc_q[sc, t], o_sb[:])

              # ================= gate tail =================
              s2f = persist.tile([128, 1], F32, tag="s2f")
              nc.vector.reduce_sum(s2f[:], s2all[:], axis=AX.X)
              ps_s2 = pst.tile([128, 1], F32, tag="pst")
              nc.tensor.matmul(ps_s2[:], ind128[:], s2f[:], start=True, stop=True)
              u0 = persist.tile([128, 1], F32, tag="u0")
              nc.vector.reciprocal(u0[:], ps_s2[:])
              u1 = persist.tile([128, 1], F32, tag="u1")
              nc.vector.tensor_tensor(u1[:], u0[:], wmask128[:], AluOpType.mult)
              u128 = persist.tile([128, 1], F32, tag="u128")
              nc.vector.tensor_scalar(u128[:], u1[:], 784.0, None, AluOpType.mult)
              u4 = persist.tile([128, 4], F16, tag="u4")
              nc.vector.tensor_tensor(u4[:], ind4[:],
                                      u128[:].broadcast_to((128, 4)),
                                      AluOpType.mult)
              wsum4 = persist.tile([4, 1], F32, tag="wsum4")
              nc.vector.reduce_sum(wsum4[:], wmaskrow4[:], axis=AX.X)
              c04 = persist.tile([4, 1], F32, tag="c04")
              nc.vector.tensor_tensor(c04[:], bmask4[:], wsum4[:],
                                      AluOpType.subtract)

              gate_v = gate_out.rearrange("(s j f) -> s j f", j=4, f=512)
              for sc in range(NSC):
                  psg = pst.tile([4, 512], F32, tag="pst")
                  nc.tensor.matmul(psg[:], u4[:], e2_tiles[sc][:],
                                   start=True, stop=True)
                  g_sb = gp.tile([4, 512], F32, tag="g")
                  nc.scalar.activation(g_sb[:], psg[:], ACT.Sigmoid,
                                       bias=c04[:, 0:1])
                  nc.sync.dma_start(gate_v[sc], g_sb[:])

    nc.compile()
    return nc


def make_in_map(b, input, color_emb, object_emb, obj2color, mask,
                W_obj, W_col, W_mask, b_mask, P=P_FULL):
    """Per-core input dict (host-side layout prep only, no math)."""
    f = np.float32
    Cc = input.shape[1]
    tgtb = np.ascontiguousarray(
        np.asarray(input[b], dtype=f).reshape(Cc, -1)[:, :P])
    mv = np.where(np.tile(np.asarray(mask[b], bool), 4),
                  np.float32(MASKNEG), np.float32(MASKPOS)).astype(f)
    W_mask = np.asarray(W_mask, dtype=f)
    return {
        "tgt": tgtb,
        "oemb": np.ascontiguousarray(np.asarray(object_emb[b], dtype=f)),
        "cemb4": np.ascontiguousarray(
            np.tile(np.asarray(color_emb[b], dtype=f), (1, 4))),
        "o2c": np.ascontiguousarray(np.asarray(obj2color[b], dtype=f)),
        "wobjT": np.ascontiguousarray(np.asarray(W_obj, dtype=f).T),
        "wcolT": np.ascontiguousarray(np.asarray(W_col, dtype=f).T),
        "maskvec": mv[:, None],
        "ind128": np.ascontiguousarray(np.tile(np.eye(Wn, dtype=f), (4, 4))),
        "ind4": np.ascontiguousarray(np.repeat(np.eye(4, dtype=f), Wn, axis=0)),
        "wmask128": np.ascontiguousarray(np.tile(W_mask[0], 4)[:, None]),
        "wmaskrow4": np.ascontiguousarray(np.tile(W_mask, (4, 1))),
        "bmask4": np.full((4, 1), np.asarray(b_mask, dtype=f).reshape(-1)[0],
                          dtype=f),
    }


_NC_CACHE = {}


def _get_nc(**kw):
    key = tuple(sorted(kw.items()))
    if key not in _NC_CACHE:
        _NC_CACHE[key] = build_nc(**kw)
    return _NC_CACHE[key]


def run(inputs, trace=False, **build_kw):
    """Run on 8 NeuronCores; returns ((wc, gate), BassKernelResults)."""
    from concourse import bass_utils

    nc = _get_nc(**build_kw)
    in_maps = [make_in_map(b, **inputs) for b in range(NCORES)]
    res = bass_utils.run_bass_kernel_spmd(
        nc, in_maps, core_ids=list(range(NCORES)), trace=trace)
    wc = np.stack([np.asarray(res.results[i]["wc"]) for i in range(NCORES)])
    gate = np.stack(
        [np.asarray(res.results[i]["gate"]) for i in range(NCORES)])
    gate = gate.reshape(B, 1, IH, IW)
    return (wc, gate), res


def kernel(input, color_emb, object_emb, obj2color, mask,
           W_obj, W_col, W_mask, b_mask):
    inputs = dict(input=input, color_emb=color_emb, object_emb=object_emb,
                  obj2color=obj2color, mask=mask, W_obj=W_obj, W_col=W_col,
                  W_mask=W_mask, b_mask=b_mask)
    (wc, gate), _ = run(inputs, trace=False)
    return wc, gate

